# revision 1
# baseline (speedup 1.0000x reference)
"""Multi-head self-attention (B=2, N=2048, C=1024, H=16, D=64) on 8 trn2 cores.

Sharding: core c handles batch b = c//4 and the 4 heads [4*(c%4), 4*(c%4)+4).
Host pre-transposes x and the weight slices; per-core partial outputs are
summed on the host and the output bias is added there.

Device kernel (per core): attention runs in bf16 (PE streams bf16 at one
row/cycle, same as fp32r, and it halves HBM/SBUF traffic); PSUM accumulation
stays fp32.
  phase 1: qkT/V projections per 512-column chunk of x; the S matmuls and
           exps of the first two attention blocks (query tile 0, both head
           pairs) are hoisted in chunk by chunk so ScalarE works during the
           projections; their exp outputs stay resident (pth) until phase 2.
  phase 2: one flat software pipeline over (query tile, head pair, j block):
           PE issues S two steps ahead of PV so ScalarE's exp stream never
           starves; the hoisted blocks' PV debt drains at up to 4 PVs/step;
           out-projection (head pairs packed so K=128) and the rank-1
           1/denominator broadcasts pop into PE slack slots as fillers, with
           a few ready units held back to cover the tail.
"""

import os
from collections import deque

import ml_dtypes
import numpy as np

import concourse.mybir as mybir
import concourse.tile as tile
from concourse import bacc
from concourse.bass_utils import run_bass_kernel_spmd

F32 = mybir.dt.float32
F32R = mybir.dt.float32r
BF16 = mybir.dt.bfloat16

B, N, C = 2, 2048, 1024
H, D = 16, 64
HPC = 4            # heads per core
P = 128
FD = 512           # matmul free-dim tile
KB = C // P        # 8 contraction blocks for the projections
NT = N // FD       # 4 free tiles over the sequence
NJB = N // P       # 16 j blocks in attention
HB = 2             # blocks (query tile 0, both prs) hoisted into phase 1

# schedule tuning knobs (env-overridable for experiments)
BGT_HI = int(os.environ.get("BGT_HI", "8"))    # PV catch-up: lag for budget 4
BGT_LO = int(os.environ.get("BGT_LO", "4"))    # PV catch-up: lag for budget 2
REP_SLOTS = tuple(int(v) for v in
                  os.environ.get("REP_SLOTS", "7,8").split(","))
FQ_LO = int(os.environ.get("FQ_LO", "9"))      # out-proj filler drain slots
FQ_HI = int(os.environ.get("FQ_HI", "13"))
PTB = int(os.environ.get("PTB", "10"))


def build_nc(repeat: int = 1) -> bacc.Bacc:
    nc = bacc.Bacc("TRN2", target_bir_lowering=False, debug=False)

    xT = nc.dram_tensor("xT", [C, N], BF16, kind="ExternalInput").ap()
    wqkvT = nc.dram_tensor("wqkvT", [C, 3 * HPC * D], BF16,
                           kind="ExternalInput").ap()
    woutT = nc.dram_tensor("woutT", [P, 2, C], BF16, kind="ExternalInput").ap()
    ones2 = nc.dram_tensor("ones2", [2, P], F32, kind="ExternalInput").ap()
    y = nc.dram_tensor("y", [N, C], BF16, kind="ExternalOutput").ap()

    xT_r = xT.rearrange("(o p) n -> p o n", p=P)          # [128, 8, 2048]
    wqkvT_r = wqkvT.rearrange("(o p) f -> p o f", p=P)    # [128, 8, 768]

    with tile.TileContext(nc) as tc:
        with (
            tc.tile_pool(name="w_pool", bufs=1) as w_pool,
            tc.tile_pool(name="qk_pool", bufs=1) as qk_pool,
            tc.tile_pool(name="v_pool", bufs=1) as v_pool,
            tc.tile_pool(name="o_pool", bufs=1) as o_pool,
            tc.tile_pool(name="x_pool", bufs=2) as x_pool,
            tc.tile_pool(name="pt_pool", bufs=PTB) as pt_pool,
            tc.tile_pool(name="pth_pool", bufs=1) as pth_pool,
            tc.tile_pool(name="y_pool", bufs=4) as y_pool,
            tc.tile_pool(name="nrm_pool", bufs=2) as nrm_pool,
            tc.tile_pool(name="ps", bufs=1, space="PSUM") as ps,
        ):
            # DMA priority order (transfers serialize through the DMA
            # engines): first x chunk halves interleaved with the wqkv weight
            # halves they unblock, then the late-needed tensors.
            wq_p = [w_pool.tile([P, 2, 3 * HPC * D], BF16, name=f"wqp{i}")
                    for i in range(4)]
            x0_p = [x_pool.tile([P, 2, FD], BF16, tag=f"x0p{i}", bufs=1,
                                name=f"x0p{i}")
                    for i in range(4)]
            for i in range(4):
                nc.sync.dma_start(wq_p[i], wqkvT_r[:, 2 * i:2 * i + 2, :])
                nc.scalar.dma_start(x0_p[i], xT_r[:, 2 * i:2 * i + 2, 0:FD])
            wo_sb = w_pool.tile([P, 2, C], BF16)
            nc.scalar.dma_start(wo_sb, woutT)
            ones2_sb = w_pool.tile([2, P], F32R)
            nc.sync.dma_start(ones2_sb, ones2.bitcast(F32R))

            qkT_sb = qk_pool.tile([P, 4, N], BF16)       # q01 | q23 | k01 | k23
            V_sb = v_pool.tile([P, NJB, HPC, D + 1], BF16)
            oT_sb = o_pool.tile([P, 2, N], BF16)         # [hh*64+d, pr, n]
            # resident exp outputs for the hoisted blocks
            pth = pth_pool.tile([P, HB * NJB, 2 * FD], BF16)
            nc.vector.memset(V_sb[:, :, :, D:D + 1], 1.0)

            blocks = [(itl, pr) for itl in range(NT) for pr in range(2)]
            TOT = len(blocks) * NJB
            HS = HB * NJB          # hoisted steps

            for _rep in range(repeat):
                fq = deque()     # out-proj filler units for PE slack slots
                repq = deque()   # 1/denominator broadcast+mul units
                tailq = []       # ready units held back to cover the tail
                ot_blk, sts, pts = {}, {}, {}

                def drain(nmax):
                    k = 0
                    while fq and k < nmax:
                        fq.popleft()()
                        k += 1

                def S_step(s):
                    (itl, pr), jb = blocks[s // NJB], s % NJB
                    st = ps.tile([P, 2 * FD], F32, tag="st", bufs=2, name="st")
                    for hh in range(2):
                        lo = hh * D
                        nc.tensor.matmul(
                            st[:, hh * FD:(hh + 1) * FD],
                            lhsT=qkT_sb[lo:lo + D, 2 + pr, jb * P:(jb + 1) * P],
                            rhs=qkT_sb[lo:lo + D, pr, itl * FD:(itl + 1) * FD],
                            start=True, stop=True)
                    sts[s] = st

                def E_step(s):
                    if s < HS:
                        pt = pth[:, s, :]
                    else:
                        pt = pt_pool.tile([P, 2 * FD], BF16, tag="pt",
                                          name="pt")
                    nc.scalar.activation(
                        pt, sts.pop(s), mybir.ActivationFunctionType.Exp,
                        scale=0.125)
                    pts[s] = pt

                def push_norm(ot_h, pr, itl, last=False):
                    """ot's PSUM readers (rin + oT evictions) come first in
                    the DVE queue so the next block's first PV (WAR on the ot
                    slot) unblocks quickly. For the final block the scalar
                    chain runs on ScalarE (idle by then), in parallel with
                    DVE's oT evictions."""
                    osl = oT_sb[:, pr, itl * FD:(itl + 1) * FD]
                    rec2s = []
                    for hh in range(2):
                        rin = nrm_pool.tile([1, FD], F32, tag="rin",
                                            name="rin")
                        if last:
                            nc.scalar.activation(
                                rin, ot_h[hh][D:D + 1, :],
                                mybir.ActivationFunctionType.Copy)
                            nc.scalar.activation(
                                osl[hh * D:(hh + 1) * D], ot_h[hh][0:D, :],
                                mybir.ActivationFunctionType.Copy)
                        else:
                            nc.vector.tensor_copy(rin, ot_h[hh][D:D + 1, :])
                            nc.vector.tensor_copy(
                                osl[hh * D:(hh + 1) * D], ot_h[hh][0:D, :])
                        rec = nrm_pool.tile([1, FD], F32, tag="rec",
                                            name="rec")
                        nc.vector.reciprocal_approx_fast(out=rec, in_=rin)
                        rec2 = nrm_pool.tile([1, FD], F32R, tag="rec2",
                                             name="rec2")
                        nc.vector.tensor_copy(rec2, rec)
                        rec2s.append(rec2)

                    def rep_mul(hh, rec2):
                        rep = ps.tile([D, FD], F32, tag="mm", bufs=2,
                                      name="rep")
                        nc.tensor.matmul(rep, lhsT=ones2_sb[0:1, 0:D],
                                         rhs=rec2, start=True, stop=True)
                        nc.vector.tensor_mul(out=osl[hh * D:(hh + 1) * D],
                                             in0=osl[hh * D:(hh + 1) * D],
                                             in1=rep)
                    for hh in range(2):
                        repq.append(lambda hh=hh, rec2=rec2s[hh]:
                                    rep_mul(hh, rec2))

                def push_outproj(itl):
                    """y rows of this query tile; heads packed so K=128. The
                    last tile's PSUM evictions go through ScalarE (idle by
                    then) so DVE isn't the tail's critical path."""
                    for i4 in range(4):
                        it = itl * 4 + i4
                        y_t = y_pool.tile([P, C], BF16, tag="yt")
                        for o2 in range(2):
                            def unit(y_t=y_t, it=it, o2=o2, itl=itl):
                                py = ps.tile([P, FD], F32, tag="mm", bufs=2,
                                             name="py")
                                for g in range(2):
                                    nc.tensor.matmul(
                                        py,
                                        lhsT=oT_sb[:, g, it * P:(it + 1) * P],
                                        rhs=wo_sb[:, g,
                                                  o2 * FD:(o2 + 1) * FD],
                                        start=(g == 0), stop=(g == 1))
                                ysl = y_t[:, o2 * FD:(o2 + 1) * FD]
                                tail_unit = (itl == NT - 1
                                             or (itl == NT - 2
                                                 and it - itl * 4 >= 2))
                                if tail_unit and (it + o2) % 2 == 0:
                                    nc.scalar.activation(
                                        ysl, py,
                                        mybir.ActivationFunctionType.Copy)
                                else:
                                    nc.vector.tensor_copy(ysl, py)
                                eng = nc.sync if (it + o2) % 2 == 0 \
                                    else nc.scalar
                                eng.dma_start(
                                    y[it * P:(it + 1) * P,
                                      o2 * FD:(o2 + 1) * FD], ysl)
                            if itl == NT - 2 and i4 >= 2:
                                tailq.append(unit)
                            else:
                                fq.append(unit)

                def PV_step(s):
                    (itl, pr), jb = blocks[s // NJB], s % NJB
                    if jb == 0:
                        ot_blk[s // NJB] = [
                            ps.tile([D + 1, FD], F32, tag="ot", bufs=2,
                                    name=f"ot{pr}{itl}{hh}") for hh in range(2)]
                    ot_h = ot_blk[s // NJB]
                    pt = pts.pop(s)
                    for hh in range(2):
                        nc.tensor.matmul(
                            ot_h[hh],
                            lhsT=V_sb[:, jb, 2 * pr + hh, :],
                            rhs=pt[:, hh * FD:(hh + 1) * FD],
                            start=(jb == 0), stop=(jb == NJB - 1))
                    if jb == NJB - 1:
                        push_norm(ot_h, pr, itl, last=(s == TOT - 1))
                        del ot_blk[s // NJB]
                        if pr == 1:
                            push_outproj(itl)

                # ---------------- phase 1: projections ----------------
                def proj_qk(nt, mt, xof):
                    pq = ps.tile([P, FD], F32, tag="mm", bufs=2, name="pq")
                    for kb in range(KB):
                        nc.tensor.matmul(
                            pq, lhsT=wq_p[kb // 2][:, kb % 2,
                                              mt * P:(mt + 1) * P],
                            rhs=xof(kb),
                            start=(kb == 0), stop=(kb == KB - 1))
                    nc.vector.tensor_copy(
                        qkT_sb[:, mt, nt * FD:(nt + 1) * FD], pq)

                def hoist_se(pr, nt):
                    for jb in range(4 * nt, 4 * nt + 4):
                        S_step(pr * NJB + jb)
                        E_step(pr * NJB + jb)

                for nt in range(NT):
                    if nt == 0:
                        xof = lambda kb: x0_p[kb // 2][:, kb % 2, :]
                    else:
                        xc = x_pool.tile([P, KB, FD], BF16, tag="xc")
                        eng = nc.sync if nt % 2 == 0 else nc.scalar
                        eng.dma_start(xc, xT_r[:, :, nt * FD:(nt + 1) * FD])
                        xof = lambda kb, xc=xc: xc[:, kb, :]
                    mts = [2, 0, 1, 3] if nt == 0 else [2, 3, 0, 1]
                    for mi, mt in enumerate(mts):
                        proj_qk(nt, mt, xof)
                        if nt == 0 and mi == 1:
                            hoist_se(0, nt)
                        elif nt == 0 and mi == 3:
                            hoist_se(1, nt)
                        elif nt > 0 and mt == 2:
                            hoist_se(0, nt)
                        elif nt > 0 and mt == 3:
                            hoist_se(1, nt)
                    for i4 in range(4):
                        it = nt * 4 + i4
                        pv = ps.tile([P, HPC * D], F32, tag="mm", bufs=2,
                                     name="pv")
                        for kb in range(KB):
                            nc.tensor.matmul(
                                pv,
                                lhsT=xof(kb)[:, i4 * P:(i4 + 1) * P],
                                rhs=wq_p[kb // 2][:, kb % 2,
                                                  2 * HPC * D:3 * HPC * D],
                                start=(kb == 0), stop=(kb == KB - 1))
                        nc.vector.tensor_copy(
                            V_sb[:, it, :, 0:D],
                            pv.rearrange("p (h d) -> p h d", d=D))

                # -------- phase 2: flat software-pipelined attention --------
                pv_ptr = 0
                for s in range(HS, TOT + 2):
                    if s < TOT:
                        S_step(s)
                    lag = s - 2 - pv_ptr
                    budget = (4 if lag > BGT_HI else
                              (2 if lag > BGT_LO else 1))
                    while pv_ptr <= min(s - 2, TOT - 1) and budget > 0:
                        PV_step(pv_ptr)
                        pv_ptr += 1
                        budget -= 1
                        # fixed drain slots per PV block: rep units pop a few
                        # steps after their push (their DVE reciprocal chain
                        # needs ~3 steps), out-proj units after that;
                        # block-boundary steps stay clear
                        m = pv_ptr % 16
                        if m in REP_SLOTS and repq:
                            repq.popleft()()
                        elif FQ_LO <= m <= FQ_HI and fq:
                            fq.popleft()()
                    if HS <= s - 1 < TOT:
                        E_step(s - 1)
                while pv_ptr < TOT:
                    PV_step(pv_ptr)
                    pv_ptr += 1
                for unit in tailq:
                    unit()
                while repq:
                    repq.popleft()()
                drain(len(fq))

    nc.finalize()
    return nc


def shard_inputs(x, w_qkv, w_out):
    """Full inputs -> list of 8 per-core input maps (host-side prep)."""
    bf = ml_dtypes.bfloat16
    x = np.asarray(x, dtype=np.float32)
    w_qkv = np.asarray(w_qkv, dtype=np.float32)
    w_out = np.asarray(w_out, dtype=np.float32)
    ones2 = np.zeros((2, P), np.float32)
    ones2[0, 0:D] = 1.0
    ones2[1, D:2 * D] = 1.0
    in_maps = []
    for c in range(8):
        b, hp = c // 4, c % 4
        rows = np.concatenate(
            [w_qkv[q * C + hp * HPC * D:(q * C + (hp + 1) * HPC * D)]
             for q in range(3)], axis=0)                      # [768, C]
        # out-proj weights packed in head pairs: [hh*64+d, pr, C]
        wo = w_out[:, hp * HPC * D:(hp + 1) * HPC * D].T      # [256, C]
        wo = wo.reshape(2, 2, D, C).transpose(1, 2, 0, 3)     # [hh, 64, pr, C]
        in_maps.append({
            "ones2": ones2,
            "xT": np.ascontiguousarray(x[b].T).astype(bf),     # [C, N]
            "wqkvT": np.ascontiguousarray(rows.T).astype(bf),  # [C, 768]
            "woutT": np.ascontiguousarray(wo.reshape(P, 2, C)).astype(bf),
        })
    return in_maps


def combine_outputs(ys, b_out):
    b_out = np.asarray(b_out, dtype=np.float32)
    ys = [np.asarray(t, dtype=np.float32) for t in ys]
    out0 = ys[0] + ys[1] + ys[2] + ys[3]
    out1 = ys[4] + ys[5] + ys[6] + ys[7]
    return np.stack([out0, out1], axis=0) + b_out[None, None, :]


_NC = None


def kernel(x, w_qkv, w_out, b_out):
    global _NC
    if _NC is None:
        _NC = build_nc()
    in_maps = shard_inputs(x, w_qkv, w_out)
    res = run_bass_kernel_spmd(_NC, in_maps, core_ids=list(range(8)))
    ys = [res.results[c]["y"] for c in range(8)]
    return combine_outputs(ys, b_out).astype(np.float32)



# revision 42
# speedup vs baseline: 1.0899x; 1.0899x over previous
"""Multi-head self-attention (B=2, N=2048, C=1024, H=16, D=64) on 8 trn2 cores.

Sharding: core c handles batch b = c//4 and the 4 heads [4*(c%4), 4*(c%4)+4).
Host pre-transposes x and the weight slices; per-core partial outputs are
summed on the host and the output bias is added there.

Device kernel (per core): attention runs in bf16 (PE streams bf16 at one
row/cycle and it halves HBM/SBUF traffic); PSUM accumulation stays fp32.
  phase 1: qkT/V projections per 512-column chunk of x; the S matmuls and
           exps of the first two attention blocks (query tile 0, both head
           pairs) are hoisted in chunk by chunk so ScalarE works during the
           projections; their exp outputs stay resident (pth) until phase 2.
  phase 2: one flat software pipeline over (query tile, head pair, j block).
           PV is computed in the transposed orientation O[q, d] (lhsT = the
           exp'd score block P^T[k, q], rhs = V[k, d]) so the PE runs at full
           K=128/M=128 utilization: each j block costs 8 matmuls of F=64
           plus 8 F=1 matmuls that accumulate the softmax denominators.
           Completed blocks are normalized with a per-query reciprocal
           (DVE tensor_scalar), transposed back to O^T via PE transpose
           (128x128 bf16, identity rhs), and fed to the out-projection
           (head pairs packed so K=128), which pops into PE slack slots.
"""

import os
from collections import deque

import ml_dtypes
import numpy as np

import concourse.mybir as mybir
import concourse.tile as tile
from concourse import bacc
from concourse.bass_utils import run_bass_kernel_spmd

F32 = mybir.dt.float32
BF16 = mybir.dt.bfloat16
COPY = mybir.ActivationFunctionType.Copy
EXP = mybir.ActivationFunctionType.Exp

B, N, C = 2, 2048, 1024
H, D = 16, 64
HPC = 4            # heads per core
P = 128
FD = 512           # matmul free-dim tile
KB = C // P        # 8 contraction blocks for the projections
NT = N // FD       # 4 free tiles over the sequence
NJB = N // P       # 16 j blocks in attention
HB = 2             # blocks (query tile 0, both prs) hoisted into phase 1

# schedule tuning knobs (env-overridable for experiments)
BGT_HI = int(os.environ.get("BGT_HI", "24"))   # PV catch-up: lag for budget 3
BGT_LO = int(os.environ.get("BGT_LO", "2"))    # PV catch-up: lag for budget 2
WRM = int(os.environ.get("WRM", "8"))          # PE p-state warmup matmuls
TR_SLOTS = tuple(int(v) for v in
                 os.environ.get("TR_SLOTS", "3,4,5,6").split(","))
FQ_LO = int(os.environ.get("FQ_LO", "7"))      # out-proj filler drain slots
FQ_HI = int(os.environ.get("FQ_HI", "14"))
PTB = int(os.environ.get("PTB", "48"))
HJ = int(os.environ.get("HJ", "16"))           # hoisted jb depth, blocks 2,3
DCAP = tuple(int(v) for v in                   # phase-1 PV drains per nt
             os.environ.get("DCAP", "0,8,4,8").split(","))


def build_nc(repeat: int = 1) -> bacc.Bacc:
    nc = bacc.Bacc("TRN2", target_bir_lowering=False, debug=False)

    xT = nc.dram_tensor("xT", [C, N], BF16, kind="ExternalInput").ap()
    # qkv weights prepacked per projection column group (q01|q23|k01|k23 and
    # v), partition-major so each piece DMAs as one contiguous 2KB run per
    # partition (small-element transfers pay a 2x DMA latency penalty)
    wqmP = nc.dram_tensor("wqmP", [P, 4, KB, P], BF16,
                          kind="ExternalInput").ap()
    wqvP = nc.dram_tensor("wqvP", [P, KB, 2 * P], BF16,
                          kind="ExternalInput").ap()
    woutT = nc.dram_tensor("woutT", [P, 2, C], BF16, kind="ExternalInput").ap()
    ident = nc.dram_tensor("ident", [P, P], BF16, kind="ExternalInput").ap()
    y = nc.dram_tensor("y", [N, C], BF16, kind="ExternalOutput").ap()

    xT_r = xT.rearrange("(o p) n -> p o n", p=P)          # [128, 8, 2048]

    with tile.TileContext(nc) as tc:
        with (
            tc.tile_pool(name="w_pool", bufs=1) as w_pool,
            tc.tile_pool(name="qk_pool", bufs=1) as qk_pool,
            tc.tile_pool(name="v_pool", bufs=1) as v_pool,
            tc.tile_pool(name="o_pool", bufs=1) as o_pool,
            tc.tile_pool(name="x_pool", bufs=2) as x_pool,
            tc.tile_pool(name="pt_pool", bufs=PTB) as pt_pool,
            tc.tile_pool(name="osb_pool", bufs=6) as osb_pool,
            tc.tile_pool(name="y_pool", bufs=4) as y_pool,
            tc.tile_pool(name="nrm_pool", bufs=3) as nrm_pool,
            tc.tile_pool(name="ps", bufs=1, space="PSUM") as ps,
        ):
            # DMA priority order (transfers serialize through the DMA
            # engines): k01 weights first, then the x0 pieces they pace
            # against, then the remaining projection columns, then the
            # late-needed tensors. Splitting wqkv by projection column gets
            # the first scores (and so ScalarE's exp stream) going ~5µs in.
            wq_m = [w_pool.tile([P, KB, P], BF16, name=f"wqm{i}")
                    for i in range(4)]                   # q01 | q23 | k01 | k23
            wq_v = w_pool.tile([P, KB, HPC * D], BF16)
            x0_p = [x_pool.tile([P, 4, FD], BF16, tag=f"x0p{i}", bufs=1,
                                name=f"x0p{i}")
                    for i in range(2)]
            nc.sync.dma_start(wq_m[2], wqmP[:, 2])
            for i in range(2):
                nc.scalar.dma_start(x0_p[i], xT_r[:, 4 * i:4 * i + 4, 0:FD])
            nc.sync.dma_start(wq_m[0], wqmP[:, 0])
            nc.sync.dma_start(wq_m[1], wqmP[:, 1])
            nc.sync.dma_start(wq_m[3], wqmP[:, 3])
            nc.sync.dma_start(wq_v, wqvP)
            wo_sb = w_pool.tile([P, 2, C], BF16)
            ident_sb = w_pool.tile([P, P], BF16)
            nc.scalar.dma_start(ident_sb, ident)
            nc.scalar.dma_start(wo_sb, woutT)
            # PE p-state warmup: the tensor engine only reaches full clock
            # after 3us of continuous execution, and the first projections are
            # DMA-paced. Chew on a zero tile so the ramp completes (and never
            # resets) before the real work lands.
            wrm_sb = w_pool.tile([P, FD], BF16)
            nc.vector.memset(wrm_sb, 0.0)

            def warm(n):
                # dummies park in the st slots, which are idle until the
                # first hoisted S step
                for _ in range(n):
                    pw = ps.tile([P, FD], F32, tag="st", bufs=2, name="pw")
                    nc.tensor.matmul(pw, lhsT=wrm_sb[:, 0:P], rhs=wrm_sb,
                                     start=True, stop=True)
            warm(WRM)

            qkT_sb = qk_pool.tile([P, 4, N], BF16)       # q01 | q23 | k01 | k23
            V_sb = v_pool.tile([P, NJB, HPC, D + 1], BF16)   # [k, jb, h, d|1]
            oT_sb = o_pool.tile([P, 2, N], BF16)         # [hh*64+d, pr, n]
            nc.vector.memset(V_sb[:, :, :, D:D + 1], 1.0)

            blocks = [(itl, pr) for itl in range(NT) for pr in range(2)]
            TOT = len(blocks) * NJB
            # hoisted steps: blocks 0,1 fully plus blocks 2,3 through jb<HJ
            hoisted = set(range(2 * NJB)) | {
                b * NJB + jb for b in (2, 3) for jb in range(HJ)}

            for _rep in range(repeat):
                fq = deque()     # out-proj filler units for PE slack slots
                transq = deque()  # deferred PE transposes (+ their oT evicts)
                tailq = deque()  # ready units held back to cover the tail
                acc_blk, sts, pts = {}, {}, {}
                # out-proj units for itl are only safe to issue once all 8 of
                # its transposes (both prs) have been issued
                tdone = {itl: 0 for itl in range(NT)}

                def S_step(s):
                    (itl, pr), jb = blocks[s // NJB], s % NJB
                    st = ps.tile([P, 2 * FD], F32, tag="st", bufs=2, name="st")
                    for hh in range(2):
                        lo = hh * D
                        nc.tensor.matmul(
                            st[:, hh * FD:(hh + 1) * FD],
                            lhsT=qkT_sb[lo:lo + D, 2 + pr, jb * P:(jb + 1) * P],
                            rhs=qkT_sb[lo:lo + D, pr, itl * FD:(itl + 1) * FD],
                            start=True, stop=True)
                    sts[s] = st

                def E_step(s):
                    pt = pt_pool.tile([P, 2 * FD], BF16, tag="pt", name="pt")
                    nc.scalar.activation(pt, sts.pop(s), EXP, scale=0.125)
                    pts[s] = pt

                def do_transpose(osb, pr, itl, qb):
                    tp = ps.tile([P, P], BF16, tag="mm", bufs=2, name="tp")
                    nc.tensor.matmul(tp, lhsT=osb, rhs=ident_sb,
                                     is_transpose=True, start=True, stop=True)
                    nc.vector.tensor_copy(
                        oT_sb[:, pr, itl * FD + qb * P:itl * FD + (qb + 1) * P],
                        tp)
                    tdone[itl] += 1

                def finish_block(bi, last=False):
                    itl, pr = blocks[bi]
                    acc = acc_blk.pop(bi)
                    rec = nrm_pool.tile([P, 8], F32, tag="rec", name="rec")
                    nc.vector.reciprocal_approx_fast(
                        out=rec,
                        in_=acc[:, :, :, D:D + 1].rearrange("p a b c -> p (a b c)"))
                    # evict region qb=0 of each bank LAST: the next block's
                    # start-matmul zeroes the whole bank and its tile dep only
                    # covers region 0, so in-order DVE must have finished the
                    # other regions by then
                    osbs = [osb_pool.tile([P, P], BF16, tag="osb", name="osb")
                            for _ in range(4)]
                    for hh in range(2):
                        for qb in (1, 2, 3, 0):
                            c = hh * 4 + qb
                            if last and hh == 1:
                                # split the final block's evictions across
                                # ScalarE (idle once exps are done) and DVE
                                nc.scalar.activation(
                                    osbs[qb][:, hh * D:(hh + 1) * D],
                                    acc[:, hh, qb, 0:D], COPY,
                                    scale=rec[:, c:c + 1])
                            else:
                                nc.vector.tensor_scalar_mul(
                                    osbs[qb][:, hh * D:(hh + 1) * D],
                                    acc[:, hh, qb, 0:D], rec[:, c:c + 1])
                    for qb in range(4):
                        transq.append(
                            lambda osb=osbs[qb], pr=pr, itl=itl, qb=qb:
                            do_transpose(osb, pr, itl, qb))
                    if pr == 1:
                        push_outproj(itl)

                def push_outproj(itl):
                    """y rows of this query tile; heads packed so K=128. The
                    last tile's PSUM evictions go through ScalarE (idle by
                    then) so DVE isn't the tail's critical path."""
                    for i4 in range(4):
                        it = itl * 4 + i4
                        y_t = y_pool.tile([P, C], BF16, tag="yt", name="yt")
                        for o2 in range(2):
                            def unit(y_t=y_t, it=it, o2=o2, itl=itl):
                                py = ps.tile([P, FD], F32, tag="mm", bufs=2,
                                             name="py")
                                for g in range(2):
                                    nc.tensor.matmul(
                                        py,
                                        lhsT=oT_sb[:, g, it * P:(it + 1) * P],
                                        rhs=wo_sb[:, g,
                                                  o2 * FD:(o2 + 1) * FD],
                                        start=(g == 0), stop=(g == 1))
                                ysl = y_t[:, o2 * FD:(o2 + 1) * FD]
                                if itl == NT - 1:
                                    # tail: alternate eviction engines and
                                    # stream each half out as soon as ready
                                    if (it + o2) % 2 == 0:
                                        nc.scalar.activation(ysl, py, COPY)
                                    else:
                                        nc.vector.tensor_copy(ysl, py)
                                    nc.sync.dma_start(
                                        y[it * P:(it + 1) * P,
                                          o2 * FD:(o2 + 1) * FD], ysl)
                                else:
                                    nc.vector.tensor_copy(ysl, py)
                                    if o2 == 1:
                                        nc.sync.dma_start(
                                            y[it * P:(it + 1) * P, :], y_t)
                            if itl == NT - 2 and i4 >= 2:
                                tailq.append((itl, unit))
                            else:
                                fq.append((itl, unit))

                def PV_step(s):
                    (itl, pr), jb = blocks[s // NJB], s % NJB
                    bi = s // NJB
                    if jb == 0:
                        # [p, hh(bank), qb, d|denom]: one accumulation group
                        # per psum bank — start zeroes the whole 2KB region,
                        # so only the first matmul into each bank starts it
                        acc_blk[bi] = ps.tile([P, 2, 4, D + 1], F32,
                                              tag="acc", bufs=1, name="acc",
                                              padded_shape=[P, 2, 4, P])
                    acc = acc_blk[bi]
                    pt = pts.pop(s)
                    for hh in range(2):
                        for qb in range(4):
                            nc.tensor.matmul(
                                acc[:, hh, qb, :],
                                lhsT=pt[:, hh * FD + qb * P:
                                        hh * FD + (qb + 1) * P],
                                rhs=V_sb[:, jb, 2 * pr + hh, :],
                                start=(jb == 0 and qb == 0),
                                stop=(jb == NJB - 1 and qb == 3))
                    if jb == NJB - 1:
                        finish_block(bi, last=(s == TOT - 1))

                # ---------------- phase 1: projections ----------------
                pv_ptr = 0

                def proj_qk(nt, mt, xof, pace=0):
                    pq = ps.tile([P, FD], F32, tag="mm", bufs=2, name="pq")
                    for kb in range(KB):
                        nc.tensor.matmul(
                            pq, lhsT=wq_m[mt][:, kb, :],
                            rhs=xof(kb),
                            start=(kb == 0), stop=(kb == KB - 1))
                        # while the x0 pieces are still streaming in, keep the
                        # PE chewing so the p-state ramp never resets
                        if pace and kb % 2 == 1 and kb < KB - 1:
                            warm(pace)
                    nc.vector.tensor_copy(
                        qkT_sb[:, mt, nt * FD:(nt + 1) * FD], pq)

                def proj_v(nt, xof):
                    for i4 in range(4):
                        it = nt * 4 + i4
                        pv = ps.tile([P, HPC * D], F32, tag="mm", bufs=2,
                                     name="pv")
                        for kb in range(KB):
                            nc.tensor.matmul(
                                pv,
                                lhsT=xof(kb)[:, i4 * P:(i4 + 1) * P],
                                rhs=wq_v[:, kb, :],
                                start=(kb == 0), stop=(kb == KB - 1))
                        nc.vector.tensor_copy(
                            V_sb[:, it, :, 0:D],
                            pv.rearrange("p (h d) -> p h d", d=D))

                def hoist(b, jb_lo, n):
                    for jb in range(jb_lo, jb_lo + n):
                        S_step(b * NJB + jb)
                        E_step(b * NJB + jb)

                def drain_pv(nmax, jb_hi):
                    nonlocal pv_ptr
                    k = 0
                    while k < nmax and pv_ptr < TOT:
                        if pv_ptr % NJB > jb_hi:   # V not projected yet
                            break
                        PV_step(pv_ptr)
                        pv_ptr += 1
                        k += 1

                xcs = {}

                def prefetch_x(nt):
                    xc = x_pool.tile([P, KB, FD], BF16, tag="xc", name="xc")
                    nc.sync.dma_start(xc, xT_r[:, :, nt * FD:(nt + 1) * FD])
                    xcs[nt] = xc

                # nt0: x0 pieces pace the k01/q01 projections; hoist the
                # first jb range of blocks 0,1 as soon as their q/k exist.
                x0of = lambda kb: x0_p[kb // 4][:, kb % 4, :]
                prefetch_x(1)
                # k01/q01 projections interleaved at x0-half granularity so
                # the PE consumes each x0 piece the moment it lands
                pq_km = [ps.tile([P, FD], F32, tag="mm", bufs=2,
                                 name=f"pq0{m}") for m in (2, 0)]
                for half in range(2):
                    for mi, mt in enumerate((2, 0)):
                        for kb in range(4 * half, 4 * half + 4):
                            nc.tensor.matmul(
                                pq_km[mi], lhsT=wq_m[mt][:, kb, :],
                                rhs=x0of(kb),
                                start=(kb == 0), stop=(kb == KB - 1))
                        if half == 0:
                            warm(1)
                for mi, mt in enumerate((2, 0)):
                    nc.vector.tensor_copy(qkT_sb[:, mt, 0:FD], pq_km[mi])
                hoist(0, 0, 4)
                proj_qk(0, 1, x0of)
                proj_qk(0, 3, x0of)
                hoist(1, 0, 4)
                proj_v(0, x0of)
                # blocks 2,3 (itl=1) hoist ranges per nt, clipped to HJ and
                # to the k-availability bound jb <= 4*nt+3
                h23 = []
                lo = 0
                for nt in range(1, NT):
                    hi = min(HJ, 4 * nt + 4, lo + (6 if nt < 3 else 16))
                    h23.append((lo, hi - lo))
                    lo = hi
                for nt in range(1, NT):
                    if nt + 1 < NT:
                        prefetch_x(nt + 1)
                    xof = lambda kb, xc=xcs[nt]: xc[:, kb, :]
                    proj_qk(nt, 2, xof)
                    hoist(0, 4 * nt, 4)
                    proj_qk(nt, 3, xof)
                    hoist(1, 4 * nt, 4)
                    proj_v(nt, xof)
                    lo23, n23 = h23[nt - 1]
                    proj_qk(nt, 0, xof)
                    if n23:
                        hoist(2, lo23, n23)
                    proj_qk(nt, 1, xof)
                    if n23:
                        hoist(3, lo23, n23)
                    drain_pv(DCAP[nt], 4 * nt + 3)

                # -------- phase 2: flat software-pipelined attention --------
                p2 = sorted(set(range(TOT)) - hoisted)
                for i in range(len(p2) + 2):
                    if i < len(p2):
                        S_step(p2[i])
                    # PVs may run through the S step exp'd 2 iterations ago
                    if i < 2:
                        cap = p2[0] - 1
                    elif i - 2 < len(p2):
                        cap = p2[i - 2]
                    else:
                        cap = TOT - 1
                    lag = cap - pv_ptr
                    budget = (3 if lag > BGT_HI else
                              (2 if lag > BGT_LO else 1))
                    def pop_fq():
                        if fq and tdone[fq[0][0]] >= 8:
                            fq.popleft()[1]()
                            return True
                        return False

                    while pv_ptr <= cap and budget > 0:
                        if pv_ptr % NJB == 0:
                            pop_fq()   # boundary: cover the acc WAR
                        PV_step(pv_ptr)
                        pv_ptr += 1
                        budget -= 1
                        # fixed drain slots per PV block: transposes pop a few
                        # steps after their block's DVE evictions, out-proj
                        # units after that; block-boundary steps stay clear
                        m = pv_ptr % 16
                        if m in TR_SLOTS and transq:
                            transq.popleft()()
                        elif FQ_LO <= m <= FQ_HI:
                            pop_fq()
                    if i >= 1 and i - 1 < len(p2):
                        E_step(p2[i - 1])
                while pv_ptr < TOT:
                    PV_step(pv_ptr)
                    pv_ptr += 1
                # tail: alternate held-back ready units with the last blocks'
                # transposes so the PE has work while the DVE chains drain
                def pop_ready(q):
                    if q and tdone[q[0][0]] >= 8:
                        q.popleft()[1]()
                        return True
                    return False

                while transq:
                    transq.popleft()()
                    if not pop_ready(tailq):
                        pop_ready(fq)
                while tailq:
                    pop_ready(tailq)
                while fq:
                    pop_ready(fq)

    nc.finalize()
    return nc


def shard_inputs(x, w_qkv, w_out):
    """Full inputs -> list of 8 per-core input maps (host-side prep)."""
    bf = ml_dtypes.bfloat16
    x = np.asarray(x, dtype=np.float32)
    w_qkv = np.asarray(w_qkv, dtype=np.float32)
    w_out = np.asarray(w_out, dtype=np.float32)
    ident = np.eye(P, dtype=np.float32)
    in_maps = []
    KB = C // P
    for c in range(8):
        b, hp = c // 4, c % 4
        rows = np.concatenate(
            [w_qkv[q * C + hp * HPC * D:(q * C + (hp + 1) * HPC * D)]
             for q in range(3)], axis=0)                      # [768, C]
        wT = rows.T.reshape(KB, P, 3 * HPC * D)               # [o, p, 768]
        # per-column-group pieces, partition-major: [p, m, o, c]
        wqm = wT[:, :, 0:4 * P].reshape(KB, P, 4, P).transpose(1, 2, 0, 3)
        wqv = wT[:, :, 4 * P:6 * P].transpose(1, 0, 2)        # [p, o, 256]
        # out-proj weights packed in head pairs: [hh*64+d, pr, C]
        wo = w_out[:, hp * HPC * D:(hp + 1) * HPC * D].T      # [256, C]
        wo = wo.reshape(2, 2, D, C).transpose(1, 2, 0, 3)     # [hh, 64, pr, C]
        in_maps.append({
            "ident": ident.astype(bf),
            "xT": np.ascontiguousarray(x[b].T).astype(bf),     # [C, N]
            "wqmP": np.ascontiguousarray(wqm).astype(bf),
            "wqvP": np.ascontiguousarray(wqv).astype(bf),
            "woutT": np.ascontiguousarray(wo.reshape(P, 2, C)).astype(bf),
        })
    return in_maps


def combine_outputs(ys, b_out):
    b_out = np.asarray(b_out, dtype=np.float32)
    ys = [np.asarray(t, dtype=np.float32) for t in ys]
    out0 = ys[0] + ys[1] + ys[2] + ys[3]
    out1 = ys[4] + ys[5] + ys[6] + ys[7]
    return np.stack([out0, out1], axis=0) + b_out[None, None, :]


_NC = None


def kernel(x, w_qkv, w_out, b_out):
    global _NC
    if _NC is None:
        _NC = build_nc()
    in_maps = shard_inputs(x, w_qkv, w_out)
    res = run_bass_kernel_spmd(_NC, in_maps, core_ids=list(range(8)))
    ys = [res.results[c]["y"] for c in range(8)]
    return combine_outputs(ys, b_out).astype(np.float32)


# revision 70
# speedup vs baseline: 1.2037x; 1.1044x over previous
"""Multi-head self-attention (B=2, N=2048, C=1024, H=16, D=64) on 8 trn2 cores.

Sharding: core c handles batch b = c//4 and the 4 heads [4*(c%4), 4*(c%4)+4).
Host pre-transposes x and the weight slices; per-core partial outputs are
summed on the host and the output bias is added there.

Device kernel (per core): attention runs in bf16 (PE streams bf16 at one
row/cycle and it halves HBM/SBUF traffic); PSUM accumulation stays fp32.
  phase 1: qkT/V projections per 512-column chunk of x; the S matmuls and
           exps of the first two attention blocks (query tile 0, both head
           pairs) are hoisted in chunk by chunk so ScalarE works during the
           projections; their exp outputs stay resident (pth) until phase 2.
  phase 2: one flat software pipeline over (query tile, head pair, j block).
           PV is computed in the transposed orientation O[q, d] (lhsT = the
           exp'd score block P^T[k, q], rhs = V[k, d]) so the PE runs at full
           K=128/M=128 utilization: each j block costs 8 matmuls of F=64
           plus 8 F=1 matmuls that accumulate the softmax denominators.
           Completed blocks are normalized with a per-query reciprocal
           (DVE tensor_scalar), transposed back to O^T via PE transpose
           (128x128 bf16, identity rhs), and fed to the out-projection
           (head pairs packed so K=128), which pops into PE slack slots.
"""

import os
from collections import deque

import ml_dtypes
import numpy as np

import concourse.mybir as mybir
import concourse.tile as tile
from concourse import bacc
from concourse.bass_utils import run_bass_kernel_spmd

F32 = mybir.dt.float32
BF16 = mybir.dt.bfloat16
COPY = mybir.ActivationFunctionType.Copy
EXP = mybir.ActivationFunctionType.Exp

B, N, C = 2, 2048, 1024
H, D = 16, 64
HPC = 4            # heads per core
P = 128
FD = 512           # matmul free-dim tile
KB = C // P        # 8 contraction blocks for the projections
NT = N // FD       # 4 free tiles over the sequence
NJB = N // P       # 16 j blocks in attention
HB = 2             # blocks (query tile 0, both prs) hoisted into phase 1

# schedule tuning knobs (env-overridable for experiments)
BGT_HI = int(os.environ.get("BGT_HI", "24"))   # PV catch-up: lag for budget 3
BGT_LO = int(os.environ.get("BGT_LO", "3"))    # PV catch-up: lag for budget 2
WRM = int(os.environ.get("WRM", "4"))          # PE p-state warmup matmuls
TR_SLOTS = tuple(int(v) for v in
                 os.environ.get("TR_SLOTS", "3,4,5,6").split(","))
FQ_LO = int(os.environ.get("FQ_LO", "8"))      # out-proj filler drain slots
FQ_HI = int(os.environ.get("FQ_HI", "15"))
PTB = int(os.environ.get("PTB", "44"))
STARVE = float(os.environ.get("STARVE", "1500"))
HJ = int(os.environ.get("HJ", "16"))           # hoisted jb depth, blocks 2,3
DCAP = tuple(int(v) for v in                   # phase-1 PV drains per nt
             os.environ.get("DCAP", "0,8,4,8").split(","))


def build_nc(repeat: int = 1) -> bacc.Bacc:
    nc = bacc.Bacc("TRN2", target_bir_lowering=False, debug=False)

    xT = nc.dram_tensor("xT", [C, N], BF16, kind="ExternalInput").ap()
    # qkv weights prepacked per projection column group (q01|q23|k01|k23 and
    # v), partition-major so each piece DMAs as one contiguous 2KB run per
    # partition (small-element transfers pay a 2x DMA latency penalty)
    wqmP = nc.dram_tensor("wqmP", [P, 4, KB, P], BF16,
                          kind="ExternalInput").ap()
    wqvP = nc.dram_tensor("wqvP", [P, KB, 2 * P], BF16,
                          kind="ExternalInput").ap()
    woutT = nc.dram_tensor("woutT", [P, 2, C], BF16, kind="ExternalInput").ap()
    ident = nc.dram_tensor("ident", [P, P], BF16, kind="ExternalInput").ap()
    y = nc.dram_tensor("y", [N, C], BF16, kind="ExternalOutput").ap()

    xT_r = xT.rearrange("(o p) n -> p o n", p=P)          # [128, 8, 2048]

    with tile.TileContext(nc) as tc:
        with (
            tc.tile_pool(name="w_pool", bufs=1) as w_pool,
            tc.tile_pool(name="qk_pool", bufs=1) as qk_pool,
            tc.tile_pool(name="v_pool", bufs=1) as v_pool,
            tc.tile_pool(name="o_pool", bufs=1) as o_pool,
            tc.tile_pool(name="x_pool", bufs=2) as x_pool,
            tc.tile_pool(name="pt_pool", bufs=PTB) as pt_pool,
            tc.tile_pool(name="osb_pool", bufs=6) as osb_pool,
            tc.tile_pool(name="y_pool", bufs=4) as y_pool,
            tc.tile_pool(name="nrm_pool", bufs=3) as nrm_pool,
            tc.tile_pool(name="ps", bufs=1, space="PSUM") as ps,
        ):
            # DMA priority order (transfers serialize through the DMA
            # engines): k01 weights first, then the x0 pieces they pace
            # against, then the remaining projection columns, then the
            # late-needed tensors. Splitting wqkv by projection column gets
            # the first scores (and so ScalarE's exp stream) going ~5µs in.
            wq_m = [w_pool.tile([P, KB, P], BF16, name=f"wqm{i}")
                    for i in range(4)]                   # q01 | q23 | k01 | k23
            wq_v = w_pool.tile([P, KB, HPC * D], BF16)
            x0_p = [x_pool.tile([P, 4, FD], BF16, tag=f"x0p{i}", bufs=1,
                                name=f"x0p{i}")
                    for i in range(2)]
            nc.sync.dma_start(wq_m[2], wqmP[:, 2])
            for i in range(2):
                nc.scalar.dma_start(x0_p[i], xT_r[:, 4 * i:4 * i + 4, 0:FD])
            nc.sync.dma_start(wq_m[0], wqmP[:, 0])
            nc.sync.dma_start(wq_m[1], wqmP[:, 1])
            nc.sync.dma_start(wq_m[3], wqmP[:, 3])
            nc.sync.dma_start(wq_v, wqvP)
            wo_sb = w_pool.tile([P, 2, C], BF16)
            ident_sb = w_pool.tile([P, P], BF16)
            nc.scalar.dma_start(ident_sb, ident)
            nc.scalar.dma_start(wo_sb, woutT)
            # PE p-state warmup: the tensor engine only reaches full clock
            # after 3us of continuous execution, and the first projections are
            # DMA-paced. Chew on a zero tile so the ramp completes (and never
            # resets) before the real work lands.
            wrm_sb = w_pool.tile([P, FD], BF16)
            nc.vector.memset(wrm_sb, 0.0)

            def warm(n):
                # dummies park in the st slots, which are idle until the
                # first hoisted S step
                for _ in range(n):
                    pw = ps.tile([P, FD], F32, tag="st", bufs=2, name="pw")
                    nc.tensor.matmul(pw, lhsT=wrm_sb[:, 0:P], rhs=wrm_sb,
                                     start=True, stop=True)
            warm(WRM)

            qkT_sb = qk_pool.tile([P, 4, N], BF16)       # q01 | q23 | k01 | k23
            V_sb = v_pool.tile([P, NJB, HPC, D + 1], BF16)   # [k, jb, h, d|1]
            oT_sb = o_pool.tile([P, 2, N], BF16)         # [hh*64+d, pr, n]
            nc.vector.memset(V_sb[:, :, :, D:D + 1], 1.0)

            blocks = [(itl, pr) for itl in range(NT) for pr in range(2)]
            TOT = len(blocks) * NJB
            # hoisted steps: blocks 0,1 fully plus blocks 2,3 through jb<HJ
            hoisted = set(range(2 * NJB)) | {
                b * NJB + jb for b in (2, 3) for jb in range(HJ)}

            for _rep in range(repeat):
                fq = deque()     # out-proj filler units for PE slack slots
                transq = deque()  # deferred PE transposes (+ their oT evicts)
                tailq = deque()  # ready units held back to cover the tail
                acc_blk, sts, pts = {}, {}, {}
                # out-proj units for itl are only safe to issue once all 8 of
                # its transposes (both prs) have been issued
                tdone = {itl: 0 for itl in range(NT)}

                def S_step(s):
                    (itl, pr), jb = blocks[s // NJB], s % NJB
                    st = ps.tile([P, 2 * FD], F32, tag="st", bufs=2, name="st")
                    for hh in range(2):
                        lo = hh * D
                        nc.tensor.matmul(
                            st[:, hh * FD:(hh + 1) * FD],
                            lhsT=qkT_sb[lo:lo + D, 2 + pr, jb * P:(jb + 1) * P],
                            rhs=qkT_sb[lo:lo + D, pr, itl * FD:(itl + 1) * FD],
                            start=True, stop=True)
                    sts[s] = st

                def E_step(s):
                    pt = pt_pool.tile([P, 2 * FD], BF16, tag="pt", name="pt")
                    nc.scalar.activation(pt, sts.pop(s), EXP, scale=0.125)
                    pts[s] = pt

                def do_transpose(osb, pr, itl, qb):
                    tp = ps.tile([P, P], BF16, tag="mm", bufs=2, name="tp")
                    nc.tensor.matmul(tp, lhsT=osb, rhs=ident_sb,
                                     is_transpose=True, start=True, stop=True)
                    nc.vector.tensor_copy(
                        oT_sb[:, pr, itl * FD + qb * P:itl * FD + (qb + 1) * P],
                        tp)
                    tdone[itl] += 1

                def finish_block(bi, last=False):
                    """Free the psum banks fast: per bank, one reciprocal of
                    the denominator column plus ONE raw (unnormalized) copy
                    to sbuf — the next block's start-matmul zeroes the whole
                    bank, so its WAR dep is just these two reads. The 8
                    per-region normalizes then run from sbuf at 2x dve rate,
                    off the accumulation critical path."""
                    itl, pr = blocks[bi]
                    accs = acc_blk.pop(bi)
                    rec = nrm_pool.tile([P, 2, 4], F32, tag="rec", name="rec")
                    raws = []
                    for hh in range(2):
                        nc.vector.reciprocal_approx_fast(
                            out=rec[:, hh, :],
                            in_=accs[hh][:, :, D:D + 1].rearrange(
                                "p a c -> p (a c)"))
                        raw = osb_pool.tile([P, 4, D], BF16, tag="raw",
                                            name="raw")
                        nc.vector.tensor_copy(raw, accs[hh][:, :, 0:D])
                        raws.append(raw)
                    osbs = [osb_pool.tile([P, P], BF16, tag="osb", name="osb")
                            for _ in range(4)]
                    for hh in range(2):
                        for qb in range(4):
                            if last and hh == 1:
                                # split the final block's normalizes across
                                # ScalarE (idle once exps are done) and DVE
                                nc.scalar.activation(
                                    osbs[qb][:, hh * D:(hh + 1) * D],
                                    raws[hh][:, qb, :], COPY,
                                    scale=rec[:, hh, qb:qb + 1])
                            else:
                                nc.vector.tensor_scalar_mul(
                                    osbs[qb][:, hh * D:(hh + 1) * D],
                                    raws[hh][:, qb, :], rec[:, hh, qb:qb + 1])
                    for qb in range(4):
                        transq.append(
                            lambda osb=osbs[qb], pr=pr, itl=itl, qb=qb:
                            do_transpose(osb, pr, itl, qb))
                    if pr == 1:
                        push_outproj(itl)

                def push_outproj(itl):
                    """y rows of this query tile; heads packed so K=128. The
                    last tile's PSUM evictions go through ScalarE (idle by
                    then) so DVE isn't the tail's critical path; its py tiles
                    also rotate through the (by then idle) acc banks so the
                    2-slot mm rotation doesn't serialize the tail."""
                    for i4 in range(4):
                        it = itl * 4 + i4
                        y_t = y_pool.tile([P, C], BF16, tag="yt", name="yt")
                        for o2 in range(2):
                            def unit(y_t=y_t, it=it, o2=o2, itl=itl):
                                if itl == NT - 1 and (2 * it + o2) % 2 == 0:
                                    tg, bf = f"acc{(it + o2) % 2}", 1
                                else:
                                    tg, bf = "mm", 2
                                py = ps.tile([P, FD], F32, tag=tg, bufs=bf,
                                             name="py")
                                for g in range(2):
                                    nc.tensor.matmul(
                                        py,
                                        lhsT=oT_sb[:, g, it * P:(it + 1) * P],
                                        rhs=wo_sb[:, g,
                                                  o2 * FD:(o2 + 1) * FD],
                                        start=(g == 0), stop=(g == 1))
                                ysl = y_t[:, o2 * FD:(o2 + 1) * FD]
                                if itl == NT - 1:
                                    # tail: alternate eviction engines and
                                    # split the DMAs across two queues
                                    if (it + o2) % 2 == 0:
                                        nc.scalar.activation(ysl, py, COPY)
                                    else:
                                        nc.vector.tensor_copy(ysl, py)
                                    eng = nc.sync if (it + o2) % 2 \
                                        else nc.gpsimd
                                    eng.dma_start(
                                        y[it * P:(it + 1) * P,
                                          o2 * FD:(o2 + 1) * FD], ysl)
                                else:
                                    nc.vector.tensor_copy(ysl, py)
                                    if o2 == 1:
                                        eng = nc.sync if it % 2 \
                                            else nc.gpsimd
                                        eng.dma_start(
                                            y[it * P:(it + 1) * P, :], y_t)
                            if itl == NT - 2 and i4 >= 2:
                                tailq.append((itl, unit))
                            else:
                                fq.append((itl, unit))

                def PV_step(s):
                    (itl, pr), jb = blocks[s // NJB], s % NJB
                    bi = s // NJB
                    if jb == 0:
                        # one psum bank (and one accumulation group) per hh:
                        # start zeroes the whole 2KB region, so only the
                        # first matmul into each bank starts it
                        acc_blk[bi] = [
                            ps.tile([P, 4, D + 1], F32, tag=f"acc{hh}",
                                    bufs=1, name=f"acc{hh}",
                                    padded_shape=[P, 4, P])
                            for hh in range(2)]
                    accs = acc_blk[bi]
                    pt = pts.pop(s)
                    for hh in range(2):
                        for qb in range(4):
                            nc.tensor.matmul(
                                accs[hh][:, qb, :],
                                lhsT=pt[:, hh * FD + qb * P:
                                        hh * FD + (qb + 1) * P],
                                rhs=V_sb[:, jb, 2 * pr + hh, :],
                                start=(jb == 0 and qb == 0),
                                stop=(jb == NJB - 1 and qb == 3))
                    if jb == NJB - 1:
                        finish_block(bi, last=(s == TOT - 1))

                # ---------------- phase 1: projections ----------------
                # Greedy emission with virtual PE/ScalarE clocks: projection
                # halves and V tiles are PE filler, hoisted S+exp pairs are
                # emitted just-in-time to keep ScalarE's exp stream fed, and
                # PV drains soak leftover PE slack. This avoids both exp
                # droughts (nt boundaries) and PE head-of-line stalls from
                # bunched S steps waiting on the st-slot rotation.
                pv_ptr = 0
                xcs = {}

                def prefetch_x(nt):
                    xc = x_pool.tile([P, KB, FD], BF16, tag="xc", name="xc")
                    nc.sync.dma_start(xc, xT_r[:, :, nt * FD:(nt + 1) * FD])
                    xcs[nt] = xc

                x0of = lambda kb: x0_p[kb // 4][:, kb % 4, :]
                prefetch_x(1)

                clk = {"pe": 5200.0, "act": 0.0}
                exp_done = []
                pq_open = {}        # (nt, mt) -> psum tile of open accum
                pv_open = {}        # nt -> psum tile
                qk_avail = {}       # (nt, mt) -> virtual time usable
                n_v = 0             # fully-emitted V tiles (gates drains)

                def emit_S(s):
                    n = len(exp_done)
                    if n >= 2:
                        clk["pe"] = max(clk["pe"], exp_done[n - 2] + 220)
                    S_step(s)
                    clk["pe"] += 427
                    e0 = max(clk["act"], clk["pe"] + 210)
                    E_step(s)
                    clk["act"] = e0 + 1038
                    exp_done.append(clk["act"])

                def emit_qk_half(nt, mt, half):
                    xof = x0of if nt == 0 else \
                        (lambda kb, xc=xcs[nt]: xc[:, kb, :])
                    if half == 0:
                        pq_open[(nt, mt)] = ps.tile([P, FD], F32, tag="mm",
                                                    bufs=2, name="pq")
                    pq = pq_open[(nt, mt)]
                    for kb in range(4 * half, 4 * half + 4):
                        nc.tensor.matmul(
                            pq, lhsT=wq_m[mt][:, kb, :], rhs=xof(kb),
                            start=(kb == 0), stop=(kb == KB - 1))
                    if nt == 0:
                        # x0 piece pacing at startup; dummies keep the PE
                        # p-state ramp from resetting in the arrival gaps
                        if half == 0 and mt in (2, 0):
                            warm(1)
                        clk["pe"] = max(clk["pe"],
                                        (5100, 6600)[half]) + 854
                    else:
                        clk["pe"] += 854
                    if half == 1:
                        nc.vector.tensor_copy(
                            qkT_sb[:, mt, nt * FD:(nt + 1) * FD],
                            pq_open.pop((nt, mt)))
                        qk_avail[(nt, mt)] = clk["pe"] + 800

                def emit_v_it(nt, i4):
                    xof = x0of if nt == 0 else \
                        (lambda kb, xc=xcs[nt]: xc[:, kb, :])
                    it = nt * 4 + i4
                    pv = ps.tile([P, HPC * D], F32, tag="mm", bufs=2,
                                 name="pv")
                    for kb in range(KB):
                        nc.tensor.matmul(
                            pv, lhsT=xof(kb)[:, i4 * P:(i4 + 1) * P],
                            rhs=wq_v[:, kb, :],
                            start=(kb == 0), stop=(kb == KB - 1))
                    nc.vector.tensor_copy(
                        V_sb[:, it, :, 0:D],
                        pv.rearrange("p (h d) -> p h d", d=D))
                    clk["pe"] += 853

                def proj_qk(nt, mt):
                    emit_qk_half(nt, mt, 0)
                    emit_qk_half(nt, mt, 1)

                def hoist(b, jb_lo, n):
                    for jb in range(jb_lo, jb_lo + n):
                        emit_S(b * NJB + jb)

                def drain_pv(nmax, jb_hi):
                    nonlocal pv_ptr
                    k = 0
                    while (k < nmax and pv_ptr < TOT
                           and pv_ptr % NJB <= jb_hi):
                        PV_step(pv_ptr)
                        pv_ptr += 1
                        k += 1

                # blocks 2,3 (itl=1) hoist ranges per nt, clipped to HJ and
                # to the k-availability bound jb <= 4*nt+3
                h23 = []
                lo = 0
                for nt in range(1, NT):
                    hi = min(HJ, 4 * nt + 4, lo + (6 if nt < 3 else 16))
                    h23.append((lo, hi - lo))
                    lo = hi

                # nt0: spread the first hoists between projections so the
                # exp stream starts as early and as evenly as possible
                proj_qk(0, 2)
                proj_qk(0, 0)
                hoist(0, 0, 2)
                proj_qk(0, 1)
                hoist(0, 2, 2)
                proj_qk(0, 3)
                hoist(1, 0, 2)
                emit_v_it(0, 0)
                emit_v_it(0, 1)
                hoist(1, 2, 2)
                emit_v_it(0, 2)
                emit_v_it(0, 3)
                for nt in range(1, NT):
                    if nt + 1 < NT:
                        prefetch_x(nt + 1)
                    proj_qk(nt, 2)
                    hoist(0, 4 * nt, 4)
                    proj_qk(nt, 3)
                    hoist(1, 4 * nt, 4)
                    for i in range(4):
                        emit_v_it(nt, i)
                    lo23, n23 = h23[nt - 1]
                    proj_qk(nt, 0)
                    if n23:
                        hoist(2, lo23, n23)
                    # drains split around the last hoist group so its exp
                    # backlog covers both the drains and the nt transition
                    drain_pv(DCAP[nt] // 2, 4 * nt + 3)
                    proj_qk(nt, 1)
                    if n23:
                        hoist(3, lo23, n23)
                    drain_pv(DCAP[nt] - DCAP[nt] // 2, 4 * nt + 3)

                # -------- phase 2: flat software-pipelined attention --------
                p2 = sorted(set(range(TOT)) - hoisted)
                for i in range(len(p2) + 2):
                    if i < len(p2):
                        S_step(p2[i])
                    # PVs may run through the S step exp'd 2 iterations ago
                    if i < 2:
                        cap = p2[0] - 1
                    elif i - 2 < len(p2):
                        cap = p2[i - 2]
                    else:
                        cap = TOT - 1
                    lag = cap - pv_ptr
                    budget = (3 if lag > BGT_HI else
                              (2 if lag > BGT_LO else 1))
                    def pop_fq():
                        if fq and tdone[fq[0][0]] >= 8:
                            fq.popleft()[1]()
                            return True
                        return False

                    while pv_ptr <= cap and budget > 0:
                        if pv_ptr % NJB == 0:
                            pop_fq()   # boundary: cover the acc WAR
                        PV_step(pv_ptr)
                        pv_ptr += 1
                        budget -= 1
                        # fixed drain slots per PV block: transposes pop a few
                        # steps after their block's DVE evictions, out-proj
                        # units after that; block-boundary steps stay clear
                        m = pv_ptr % 16
                        if m in TR_SLOTS and transq:
                            transq.popleft()()
                        elif FQ_LO <= m <= FQ_HI:
                            if not pop_fq() and tailq \
                                    and tdone[tailq[0][0]] >= 8:
                                tailq.popleft()[1]()
                    if i >= 1 and i - 1 < len(p2):
                        E_step(p2[i - 1])
                while pv_ptr < TOT:
                    PV_step(pv_ptr)
                    pv_ptr += 1
                # tail: alternate held-back ready units with the last blocks'
                # transposes so the PE has work while the DVE chains drain
                def pop_ready(q):
                    if q and tdone[q[0][0]] >= 8:
                        q.popleft()[1]()
                        return True
                    return False

                while transq:
                    transq.popleft()()
                    if not pop_ready(tailq):
                        pop_ready(fq)
                while tailq:
                    pop_ready(tailq)
                while fq:
                    pop_ready(fq)

    nc.finalize()
    return nc


def shard_inputs(x, w_qkv, w_out):
    """Full inputs -> list of 8 per-core input maps (host-side prep)."""
    bf = ml_dtypes.bfloat16
    x = np.asarray(x, dtype=np.float32)
    w_qkv = np.asarray(w_qkv, dtype=np.float32)
    w_out = np.asarray(w_out, dtype=np.float32)
    ident = np.eye(P, dtype=np.float32)
    in_maps = []
    KB = C // P
    for c in range(8):
        b, hp = c // 4, c % 4
        rows = np.concatenate(
            [w_qkv[q * C + hp * HPC * D:(q * C + (hp + 1) * HPC * D)]
             for q in range(3)], axis=0)                      # [768, C]
        wT = rows.T.reshape(KB, P, 3 * HPC * D)               # [o, p, 768]
        # per-column-group pieces, partition-major: [p, m, o, c]
        wqm = wT[:, :, 0:4 * P].reshape(KB, P, 4, P).transpose(1, 2, 0, 3)
        wqv = wT[:, :, 4 * P:6 * P].transpose(1, 0, 2)        # [p, o, 256]
        # out-proj weights packed in head pairs: [hh*64+d, pr, C]
        wo = w_out[:, hp * HPC * D:(hp + 1) * HPC * D].T      # [256, C]
        wo = wo.reshape(2, 2, D, C).transpose(1, 2, 0, 3)     # [hh, 64, pr, C]
        in_maps.append({
            "ident": ident.astype(bf),
            "xT": np.ascontiguousarray(x[b].T).astype(bf),     # [C, N]
            "wqmP": np.ascontiguousarray(wqm).astype(bf),
            "wqvP": np.ascontiguousarray(wqv).astype(bf),
            "woutT": np.ascontiguousarray(wo.reshape(P, 2, C)).astype(bf),
        })
    return in_maps


def combine_outputs(ys, b_out):
    b_out = np.asarray(b_out, dtype=np.float32)
    ys = [np.asarray(t, dtype=np.float32) for t in ys]
    out0 = ys[0] + ys[1] + ys[2] + ys[3]
    out1 = ys[4] + ys[5] + ys[6] + ys[7]
    return np.stack([out0, out1], axis=0) + b_out[None, None, :]


_NC = None


def kernel(x, w_qkv, w_out, b_out):
    global _NC
    if _NC is None:
        _NC = build_nc()
    in_maps = shard_inputs(x, w_qkv, w_out)
    res = run_bass_kernel_spmd(_NC, in_maps, core_ids=list(range(8)))
    ys = [res.results[c]["y"] for c in range(8)]
    return combine_outputs(ys, b_out).astype(np.float32)
